# revision 12
# baseline (speedup 1.0000x reference)
"""Multi-head attention (B=2, S=2048, D=1024, H=16) on 8 Trainium2 NeuronCores.

Sharding: data-parallel over batch (4 cores per batch element) x tensor-parallel
over heads (4 heads per core). Each core:
  - projects q/k/v (transposed layouts) for its 256-column slice of Wq/Wk/Wv
  - computes scores^T tiles [j,i] per head (fp32r matmuls), exp on ScalarE
  - row-sums fused into the context matmul via a ones-column appended to V
  - normalizes attn in place, writes attn^T slices to HBM
  - projects context through its 256-row slice of Wo -> partial out
Host: sums the 4 partial outs per batch (+bo) and transposes attn^T -> attn.

The math is exact to the reference modulo fp32r (~tf32) matmul rounding.
"""
import numpy as np

import concourse.tile as tile
from concourse import bacc, mybir
from concourse.bass_utils import run_bass_kernel_spmd

F32 = mybir.dt.float32
F32R = mybir.dt.float32r
AFT = mybir.ActivationFunctionType

B, S, D, H, HD = 2, 2048, 1024, 16, 64
NC = 8
GPB = NC // B            # 4 cores per batch
HPC = H // GPB           # 4 heads per core
COLS = HPC * HD          # 256 columns of the D dim per core
NM = COLS // 128         # 2 output slabs for Q/K projections
NDC = D // 128           # 8 contraction chunks
SCH = 512                # phase-1 S chunk
NSCH = S // SCH
QCH = 512                # phase-2 i chunk
NQ = S // QCH
NJT = S // 128           # 16 j tiles
SCALE = 0.125            # 1/sqrt(HD)

_NC_CACHE = []
VARIANT = "base"


def _build(reps=1):
    nc = bacc.Bacc("TRN2", target_bir_lowering=False, num_devices=NC)

    qT = nc.dram_tensor("qT", [D, S], F32R, kind="ExternalInput")
    kT = nc.dram_tensor("kT", [D, S], F32R, kind="ExternalInput")
    vT = nc.dram_tensor("vT", [D, S], F32R, kind="ExternalInput")
    wq = nc.dram_tensor("wq", [D, COLS], F32R, kind="ExternalInput")
    wk = nc.dram_tensor("wk", [D, COLS], F32R, kind="ExternalInput")
    wv = nc.dram_tensor("wv", [D, COLS], F32R, kind="ExternalInput")
    wo = nc.dram_tensor("wo", [COLS, D], F32R, kind="ExternalInput")
    bq = nc.dram_tensor("bq", [NM, 128, 1], F32, kind="ExternalInput")
    bk = nc.dram_tensor("bk", [NM, 128, 1], F32, kind="ExternalInput")
    bvr = nc.dram_tensor("bvr", [1, COLS], F32R, kind="ExternalInput")

    attnT_out = nc.dram_tensor("attnT_out", [HPC, S, S], F32, kind="ExternalOutput")
    outp = nc.dram_tensor("outp", [S, D], F32, kind="ExternalOutput")

    with tile.TileContext(nc) as tc:
        def emit_body():
            emit_once(nc, tc, locals_d)
        locals_d = dict(
            qT=qT, kT=kT, vT=vT, wq=wq, wk=wk, wv=wv, wo=wo,
            bq=bq, bk=bk, bvr=bvr, attnT_out=attnT_out, outp=outp,
        )
        if reps == 1:
            emit_body()
        else:
            with tc.For_i(0, reps, 1):
                emit_body()

    nc.compile()
    return nc


def emit_once(nc, tc, t):
    qT, kT, vT = t["qT"], t["kT"], t["vT"]
    wq, wk, wv, wo = t["wq"], t["wk"], t["wv"], t["wo"]
    bq, bk, bvr = t["bq"], t["bk"], t["bvr"]
    attnT_out, outp = t["attnT_out"], t["outp"]
    if True:
        with tc.tile_pool(name="persist", bufs=1) as persist:
            qt_sb = persist.tile([128, NM, S], F32R, tag="qt_sb")
            kt_sb = persist.tile([128, NM, S], F32R, tag="kt_sb")
            # V' per head: [j-part, jt, head, 64 v cols + 1 ones col]
            vp_sb = persist.tile([128, NJT, HPC, HD + 1], F32R, tag="vp_sb")
            ctx_sb = [
                persist.tile([HD, S], F32R, tag=f"ctx{h}", name=f"ctx{h}")
                for h in range(HPC)
            ]
            wo_sb = persist.tile([HD, HPC, D], F32R, tag="wo_sb")

            # V' ones column: memset fp32 staging then round-copy to f32r
            with tc.tile_pool(name="const_stage", bufs=1) as cstage:
                stv = cstage.tile([128, NJT, HPC, 1], F32, tag="stv")
                nc.any.memset(stv[:], 1.0)
                nc.vector.tensor_copy(vp_sb[:, :, :, HD : HD + 1], stv[:])

            # ---------------- phase 1: projections ----------------
            with (
                tc.tile_pool(name="w1", bufs=1) as w1,
                tc.tile_pool(name="x1", bufs=2) as x1,
                tc.tile_pool(name="ps1", bufs=2, space="PSUM") as ps1,
            ):
                wq_sb = w1.tile([128, NDC, COLS], F32R, tag="wq_sb")
                wk_sb = w1.tile([128, NDC, COLS], F32R, tag="wk_sb")
                wv_sb = w1.tile([128, NDC, COLS], F32R, tag="wv_sb")
                bq_sb = w1.tile([128, NM, 1], F32, tag="bq_sb")
                bk_sb = w1.tile([128, NM, 1], F32, tag="bk_sb")
                bv_row = w1.tile([1, COLS], F32R, tag="bv_row")
                # SP: wk, bk, K chunks, Q chunk 0 (everything unit 0 needs)
                # Pool: wv, bvr, bq, V chunks, wq, Q chunks 1-3, wo
                nc.sync.dma_start(wk_sb[:], wk.rearrange("(c p) m -> p c m", p=128))
                nc.sync.dma_start(bk_sb[:], bk.rearrange("m p one -> p m one"))
                nc.gpsimd.dma_start(wv_sb[:], wv.rearrange("(c p) m -> p c m", p=128))
                nc.gpsimd.dma_start(bv_row[:], bvr[:])
                nc.gpsimd.dma_start(bq_sb[:], bq.rearrange("m p one -> p m one"))
                ones128 = w1.tile([1, 128], F32R, tag="ones128")
                st1 = w1.tile([1, 128], F32, tag="st1")
                nc.any.memset(st1[:], 1.0)
                nc.vector.tensor_copy(ones128[:], st1[:])

                def qk_proj(xdram, w_sb, b_sb, out_sb, dma_engs, xtag="xin"):
                    for sc in range(NSCH):
                        x_sb = x1.tile([128, NDC, SCH], F32R, tag=xtag, name="x_sb")
                        dma_engs[sc % len(dma_engs)].dma_start(
                            x_sb[:],
                            xdram.rearrange("(c p) s -> p c s", p=128)[
                                :, :, sc * SCH : (sc + 1) * SCH
                            ],
                        )
                        pq = ps1.tile([128, NM, SCH], F32, tag="pproj", name="pq")
                        for m in range(NM):
                            for c in range(NDC):
                                nc.tensor.matmul(
                                    pq[:, m, :],
                                    w_sb[:, c, m * 128 : (m + 1) * 128],
                                    x_sb[:, c, :],
                                    start=(c == 0),
                                    stop=(c == NDC - 1),
                                )
                        for m in range(NM):
                            nc.scalar.activation(
                                out_sb[:, m, sc * SCH : (sc + 1) * SCH],
                                pq[:, m, :],
                                AFT.Identity,
                                bias=b_sb[:, m, :],
                            )

                # K first (scores need all of K), then V, then Q.
                # K rides HWDGE (SP) while V/Q ride SWDGE (Pool) in parallel.
                qk_proj(kT, wk_sb, bk_sb, kt_sb, (nc.sync,))

                # V -> natural layout [j, d] with bias via K=1 matmul
                for sc in range(NSCH):
                    v_sb = x1.tile([128, NDC, SCH], F32R, tag="xin", name="v_sb")
                    nc.gpsimd.dma_start(
                        v_sb[:],
                        vT.rearrange("(c p) s -> p c s", p=128)[
                            :, :, sc * SCH : (sc + 1) * SCH
                        ],
                    )
                    pv = ps1.tile([128, SCH // 128, COLS], F32, tag="pv", name="pv")
                    for jj in range(SCH // 128):
                        for c in range(NDC):
                            nc.tensor.matmul(
                                pv[:, jj, :],
                                v_sb[:, c, jj * 128 : (jj + 1) * 128],
                                wv_sb[:, c, :],
                                start=(c == 0),
                                stop=False,
                            )
                        nc.tensor.matmul(
                            pv[:, jj, :],
                            ones128[:, 0:128],
                            bv_row[:],
                            start=False,
                            stop=True,
                        )
                        jt = sc * (SCH // 128) + jj
                        # scatter [128, COLS] -> per-head columns of V'
                        nc.scalar.activation(
                            vp_sb[:, jt, :, 0:HD],
                            pv[:, jj, :].rearrange("p (h d) -> p h d", h=HPC),
                            AFT.Copy,
                        )

                nc.gpsimd.dma_start(wq_sb[:], wq.rearrange("(c p) m -> p c m", p=128))
                qk_proj(qT, wq_sb, bq_sb, qt_sb, (nc.sync,), xtag="xq")
                nc.gpsimd.dma_start(wo_sb[:], wo.rearrange("(h p) d -> p h d", p=HD))

            # ---------- phase 2: attention + inline output projection ----------
            with (
                tc.tile_pool(name="p2sb", bufs=2) as p2sb,
                tc.tile_pool(name="p2small", bufs=3) as p2small,
                tc.tile_pool(name="p3sb", bufs=2) as p3sb,
                tc.tile_pool(name="pscore", bufs=2, space="PSUM") as pscore_pool,
                tc.tile_pool(name="pctx", bufs=2, space="PSUM") as pctx_pool,
                tc.tile_pool(name="pout", bufs=2, space="PSUM") as pout_pool,
            ):
                for q in range(NQ):
                    qsl = slice(q * QCH, (q + 1) * QCH)
                    for h in range(HPC):
                        m, r0 = h // 2, (h % 2) * 64
                        texp = p2sb.tile([128, NJT, QCH], F32R, tag="texp")
                        pctx = pctx_pool.tile([HD + 1, QCH], F32, tag="pctx")
                        for g in range(NJT // 2):
                            ps = pscore_pool.tile([128, 2, QCH], F32, tag="ps")
                            for u in range(2):
                                jt = 2 * g + u
                                nc.tensor.matmul(
                                    ps[:, u, :],
                                    kt_sb[r0 : r0 + 64, m, jt * 128 : (jt + 1) * 128],
                                    qt_sb[r0 : r0 + 64, m, qsl],
                                    start=True,
                                    stop=True,
                                )
                            nc.scalar.activation(
                                texp[:, 2 * g : 2 * g + 2, :],
                                ps[:],
                                AFT.Exp,
                                scale=SCALE,
                            )
                            for u in range(2):
                                jt = 2 * g + u
                                nc.tensor.matmul(
                                    pctx[:],
                                    vp_sb[:, jt, h, :],
                                    texp[:, jt, :],
                                    start=(jt == 0),
                                    stop=(jt == NJT - 1),
                                )
                        # reciprocal of row sums, broadcast across partitions
                        trec1 = p2small.tile([1, QCH], F32, tag="trec1")
                        nc.vector.reciprocal(trec1[:], pctx[HD : HD + 1, :])
                        trec = p2small.tile([128, 1, QCH], F32, tag="trec")
                        nc.gpsimd.partition_broadcast(trec[:, 0, :], trec1[:])
                        # normalize attn^T in place (f32r out = rounding write)
                        if VARIANT == "dvenorm":
                            nc.vector.tensor_mul(
                                texp[:],
                                texp[:].bitcast(F32),
                                trec[:].broadcast_to((128, NJT, QCH)),
                            )
                        else:
                            HALF = NJT // 2
                            nc.vector.tensor_mul(
                                texp[:, 0:HALF, :],
                                texp[:, 0:HALF, :].bitcast(F32),
                                trec[:].broadcast_to((128, HALF, QCH)),
                            )
                            nc.gpsimd.tensor_mul(
                                texp[:, HALF:NJT, :],
                                texp[:, HALF:NJT, :].bitcast(F32),
                                trec[:].broadcast_to((128, HALF, QCH)),
                            )
                        # normalized context slice for this head
                        nc.vector.tensor_mul(
                            ctx_sb[h][:, qsl], pctx[0:HD, :], trec[0:HD, 0, :]
                        )
                        attn_dst = attnT_out[h].rearrange(
                            "(jt p) i -> p jt i", p=128
                        )[:, :, qsl]
                        if VARIANT == "dmasplit":
                            HJ = NJT // 2
                            nc.sync.dma_start(
                                attn_dst[:, 0:HJ, :], texp[:, 0:HJ, :].bitcast(F32)
                            )
                            nc.scalar.dma_start(
                                attn_dst[:, HJ:NJT, :],
                                texp[:, HJ:NJT, :].bitcast(F32),
                            )
                        else:
                            nc.sync.dma_start(attn_dst, texp[:].bitcast(F32))

                    # output projection for this quarter's i-tiles
                    for jj in range(QCH // 128):
                        it = q * (QCH // 128) + jj
                        isl = slice(it * 128, (it + 1) * 128)
                        o_sb = p3sb.tile([128, D], F32, tag="osb")
                        for nh in range(D // 512):
                            po = pout_pool.tile([128, 512], F32, tag="po")
                            for h in range(HPC):
                                nc.tensor.matmul(
                                    po[:],
                                    ctx_sb[h][:, isl],
                                    wo_sb[:, h, nh * 512 : (nh + 1) * 512],
                                    start=(h == 0),
                                    stop=(h == HPC - 1),
                                )
                            nc.scalar.activation(
                                o_sb[:, nh * 512 : (nh + 1) * 512], po[:], AFT.Copy
                            )
                        nc.gpsimd.dma_start(outp[isl, :], o_sb[:])


def _get_nc():
    if not _NC_CACHE:
        _NC_CACHE.append(_build())
    return _NC_CACHE[0]


def kernel(**inputs):
    q = np.ascontiguousarray(np.asarray(inputs["q"], dtype=np.float32))
    k = np.ascontiguousarray(np.asarray(inputs["k"], dtype=np.float32))
    v = np.ascontiguousarray(np.asarray(inputs["v"], dtype=np.float32))
    Wq = np.asarray(inputs["Wq"], dtype=np.float32)
    Wk = np.asarray(inputs["Wk"], dtype=np.float32)
    Wv = np.asarray(inputs["Wv"], dtype=np.float32)
    Wo = np.asarray(inputs["Wo"], dtype=np.float32)
    bq = np.asarray(inputs["bq"], dtype=np.float32)
    bk = np.asarray(inputs["bk"], dtype=np.float32)
    bv = np.asarray(inputs["bv"], dtype=np.float32)
    bo = np.asarray(inputs["bo"], dtype=np.float32)

    nc = _get_nc()

    qTs = [np.ascontiguousarray(q[b].T) for b in range(B)]
    kTs = [np.ascontiguousarray(k[b].T) for b in range(B)]
    vTs = [np.ascontiguousarray(v[b].T) for b in range(B)]

    in_maps = []
    for core in range(NC):
        b, hb = divmod(core, GPB)
        col0 = hb * COLS
        csl = slice(col0, col0 + COLS)
        in_maps.append(
            {
                "qT": qTs[b],
                "kT": kTs[b],
                "vT": vTs[b],
                "wq": np.ascontiguousarray(Wq[:, csl]),
                "wk": np.ascontiguousarray(Wk[:, csl]),
                "wv": np.ascontiguousarray(Wv[:, csl]),
                "wo": np.ascontiguousarray(Wo[csl, :]),
                "bq": np.ascontiguousarray(bq[csl]).reshape(NM, 128, 1),
                "bk": np.ascontiguousarray(bk[csl]).reshape(NM, 128, 1),
                "bvr": np.ascontiguousarray(bv[csl]).reshape(1, COLS),
            }
        )

    res = run_bass_kernel_spmd(nc, in_maps, core_ids=list(range(NC)))

    attn = np.empty((B, H, S, S), dtype=np.float32)
    out = np.zeros((B, S, D), dtype=np.float32)
    for core in range(NC):
        b, hb = divmod(core, GPB)
        r = res.results[core]
        part = r["attnT_out"]
        for hl in range(HPC):
            attn[b, hb * HPC + hl] = part[hl].T
        out[b] += r["outp"]
    out += bo
    return out, attn


# revision 14
# speedup vs baseline: 1.0655x; 1.0655x over previous
"""Multi-head attention (B=2, S=2048, D=1024, H=16) on 8 Trainium2 NeuronCores.

Sharding: data-parallel over batch (4 cores per batch element) x tensor-parallel
over heads (4 heads per core). Each core:
  - projects q/k/v (transposed layouts) for its 256-column slice of Wq/Wk/Wv
  - computes scores^T tiles [j,i] per head (fp32r matmuls), exp on ScalarE
  - row-sums fused into the context matmul via a ones-column appended to V
  - normalizes attn in place, writes attn^T slices to HBM
  - projects context through its 256-row slice of Wo -> partial out
Host: sums the 4 partial outs per batch (+bo) and transposes attn^T -> attn.

The math is exact to the reference modulo fp32r (~tf32) matmul rounding.
"""
import numpy as np

import concourse.tile as tile
from concourse import bacc, mybir
from concourse.bass_utils import run_bass_kernel_spmd

F32 = mybir.dt.float32
F32R = mybir.dt.float32r
AFT = mybir.ActivationFunctionType

B, S, D, H, HD = 2, 2048, 1024, 16, 64
NC = 8
GPB = NC // B            # 4 cores per batch
HPC = H // GPB           # 4 heads per core
COLS = HPC * HD          # 256 columns of the D dim per core
NM = COLS // 128         # 2 output slabs for Q/K projections
NDC = D // 128           # 8 contraction chunks
SCH = 512                # phase-1 S chunk
NSCH = S // SCH
QCH = 512                # phase-2 i chunk
NQ = S // QCH
NJT = S // 128           # 16 j tiles
SCALE = 0.125            # 1/sqrt(HD)

_NC_CACHE = []
VARIANT = "base"


def _build(reps=1):
    nc = bacc.Bacc("TRN2", target_bir_lowering=False, num_devices=NC)

    qT = nc.dram_tensor("qT", [D, S], F32R, kind="ExternalInput")
    kT = nc.dram_tensor("kT", [D, S], F32R, kind="ExternalInput")
    vT = nc.dram_tensor("vT", [D, S], F32R, kind="ExternalInput")
    wq = nc.dram_tensor("wq", [D, COLS], F32R, kind="ExternalInput")
    wk = nc.dram_tensor("wk", [D, COLS], F32R, kind="ExternalInput")
    wv = nc.dram_tensor("wv", [D, COLS], F32R, kind="ExternalInput")
    wo = nc.dram_tensor("wo", [COLS, D], F32R, kind="ExternalInput")
    bq = nc.dram_tensor("bq", [NM, 128, 1], F32, kind="ExternalInput")
    bk = nc.dram_tensor("bk", [NM, 128, 1], F32, kind="ExternalInput")
    bvr = nc.dram_tensor("bvr", [1, COLS], F32R, kind="ExternalInput")

    if VARIANT == "ctg":
        attnT_out = nc.dram_tensor(
            "attnT_out", [HPC, NQ, 128, NJT, QCH], F32, kind="ExternalOutput"
        )
    else:
        attnT_out = nc.dram_tensor(
            "attnT_out", [HPC, S, S], F32, kind="ExternalOutput"
        )
    outp = nc.dram_tensor("outp", [S, D], F32, kind="ExternalOutput")

    with tile.TileContext(nc) as tc:
        def emit_body():
            emit_once(nc, tc, locals_d)
        locals_d = dict(
            qT=qT, kT=kT, vT=vT, wq=wq, wk=wk, wv=wv, wo=wo,
            bq=bq, bk=bk, bvr=bvr, attnT_out=attnT_out, outp=outp,
        )
        if reps == 1:
            emit_body()
        else:
            with tc.For_i(0, reps, 1):
                emit_body()

    nc.compile()
    return nc


def emit_once(nc, tc, t):
    qT, kT, vT = t["qT"], t["kT"], t["vT"]
    wq, wk, wv, wo = t["wq"], t["wk"], t["wv"], t["wo"]
    bq, bk, bvr = t["bq"], t["bk"], t["bvr"]
    attnT_out, outp = t["attnT_out"], t["outp"]
    if True:
        with tc.tile_pool(name="persist", bufs=1) as persist:
            qt_sb = persist.tile([128, NM, S], F32R, tag="qt_sb")
            kt_sb = persist.tile([128, NM, S], F32R, tag="kt_sb")
            # V' per head: [j-part, jt, head, 64 v cols + 1 ones col]
            vp_sb = persist.tile([128, NJT, HPC, HD + 1], F32R, tag="vp_sb")
            ctx_sb = [
                persist.tile([HD, S], F32R, tag=f"ctx{h}", name=f"ctx{h}")
                for h in range(HPC)
            ]
            wo_sb = persist.tile([HD, HPC, D], F32R, tag="wo_sb")

            # V' ones column: memset fp32 staging then round-copy to f32r
            with tc.tile_pool(name="const_stage", bufs=1) as cstage:
                stv = cstage.tile([128, NJT, HPC, 1], F32, tag="stv")
                nc.any.memset(stv[:], 1.0)
                nc.vector.tensor_copy(vp_sb[:, :, :, HD : HD + 1], stv[:])

            # ---------------- phase 1: projections ----------------
            with (
                tc.tile_pool(name="w1", bufs=1) as w1,
                tc.tile_pool(name="x1", bufs=2) as x1,
                tc.tile_pool(name="ps1", bufs=2, space="PSUM") as ps1,
            ):
                wq_sb = w1.tile([128, NDC, COLS], F32R, tag="wq_sb")
                wk_sb = w1.tile([128, NDC, COLS], F32R, tag="wk_sb")
                wv_sb = w1.tile([128, NDC, COLS], F32R, tag="wv_sb")
                bq_sb = w1.tile([128, NM, 1], F32, tag="bq_sb")
                bk_sb = w1.tile([128, NM, 1], F32, tag="bk_sb")
                bv_row = w1.tile([1, COLS], F32R, tag="bv_row")
                # SP: wk, bk, K chunks, Q chunk 0 (everything unit 0 needs)
                # Pool: wv, bvr, bq, V chunks, wq, Q chunks 1-3, wo
                nc.sync.dma_start(wk_sb[:], wk.rearrange("(c p) m -> p c m", p=128))
                nc.sync.dma_start(bk_sb[:], bk.rearrange("m p one -> p m one"))
                nc.gpsimd.dma_start(wv_sb[:], wv.rearrange("(c p) m -> p c m", p=128))
                nc.gpsimd.dma_start(bv_row[:], bvr[:])
                nc.gpsimd.dma_start(bq_sb[:], bq.rearrange("m p one -> p m one"))
                ones128 = w1.tile([1, 128], F32R, tag="ones128")
                st1 = w1.tile([1, 128], F32, tag="st1")
                nc.any.memset(st1[:], 1.0)
                nc.vector.tensor_copy(ones128[:], st1[:])

                def qk_proj(xdram, w_sb, b_sb, out_sb, dma_engs, xtag="xin"):
                    for sc in range(NSCH):
                        x_sb = x1.tile([128, NDC, SCH], F32R, tag=xtag, name="x_sb")
                        dma_engs[sc % len(dma_engs)].dma_start(
                            x_sb[:],
                            xdram.rearrange("(c p) s -> p c s", p=128)[
                                :, :, sc * SCH : (sc + 1) * SCH
                            ],
                        )
                        pq = ps1.tile([128, NM, SCH], F32, tag="pproj", name="pq")
                        for m in range(NM):
                            for c in range(NDC):
                                nc.tensor.matmul(
                                    pq[:, m, :],
                                    w_sb[:, c, m * 128 : (m + 1) * 128],
                                    x_sb[:, c, :],
                                    start=(c == 0),
                                    stop=(c == NDC - 1),
                                )
                        for m in range(NM):
                            nc.scalar.activation(
                                out_sb[:, m, sc * SCH : (sc + 1) * SCH],
                                pq[:, m, :],
                                AFT.Identity,
                                bias=b_sb[:, m, :],
                            )

                # K first (scores need all of K), then V, then Q.
                # K rides HWDGE (SP) while V/Q ride SWDGE (Pool) in parallel.
                qk_proj(kT, wk_sb, bk_sb, kt_sb, (nc.sync,))

                # V -> natural layout [j, d] with bias via K=1 matmul
                for sc in range(NSCH):
                    v_sb = x1.tile([128, NDC, SCH], F32R, tag="xin", name="v_sb")
                    nc.gpsimd.dma_start(
                        v_sb[:],
                        vT.rearrange("(c p) s -> p c s", p=128)[
                            :, :, sc * SCH : (sc + 1) * SCH
                        ],
                    )
                    pv = ps1.tile([128, SCH // 128, COLS], F32, tag="pv", name="pv")
                    for jj in range(SCH // 128):
                        for c in range(NDC):
                            nc.tensor.matmul(
                                pv[:, jj, :],
                                v_sb[:, c, jj * 128 : (jj + 1) * 128],
                                wv_sb[:, c, :],
                                start=(c == 0),
                                stop=False,
                            )
                        nc.tensor.matmul(
                            pv[:, jj, :],
                            ones128[:, 0:128],
                            bv_row[:],
                            start=False,
                            stop=True,
                        )
                        jt = sc * (SCH // 128) + jj
                        # scatter [128, COLS] -> per-head columns of V'
                        nc.scalar.activation(
                            vp_sb[:, jt, :, 0:HD],
                            pv[:, jj, :].rearrange("p (h d) -> p h d", h=HPC),
                            AFT.Copy,
                        )

                nc.gpsimd.dma_start(wq_sb[:], wq.rearrange("(c p) m -> p c m", p=128))
                qk_proj(qT, wq_sb, bq_sb, qt_sb, (nc.sync,), xtag="xq")
                nc.gpsimd.dma_start(wo_sb[:], wo.rearrange("(h p) d -> p h d", p=HD))

            # ---------- phase 2: attention + inline output projection ----------
            with (
                tc.tile_pool(name="p2sb", bufs=2) as p2sb,
                tc.tile_pool(name="p2small", bufs=3) as p2small,
                tc.tile_pool(name="p3sb", bufs=2) as p3sb,
                tc.tile_pool(name="pscore", bufs=2, space="PSUM") as pscore_pool,
                tc.tile_pool(name="pctx", bufs=2, space="PSUM") as pctx_pool,
                tc.tile_pool(name="pout", bufs=2, space="PSUM") as pout_pool,
            ):
                for q in range(NQ):
                    qsl = slice(q * QCH, (q + 1) * QCH)
                    for h in range(HPC):
                        m, r0 = h // 2, (h % 2) * 64
                        texp = p2sb.tile([128, NJT, QCH], F32R, tag="texp")
                        pctx = pctx_pool.tile([HD + 1, QCH], F32, tag="pctx")
                        for g in range(NJT // 2):
                            ps = pscore_pool.tile([128, 2, QCH], F32, tag="ps")
                            for u in range(2):
                                jt = 2 * g + u
                                nc.tensor.matmul(
                                    ps[:, u, :],
                                    kt_sb[r0 : r0 + 64, m, jt * 128 : (jt + 1) * 128],
                                    qt_sb[r0 : r0 + 64, m, qsl],
                                    start=True,
                                    stop=True,
                                )
                            nc.scalar.activation(
                                texp[:, 2 * g : 2 * g + 2, :],
                                ps[:],
                                AFT.Exp,
                                scale=SCALE,
                            )
                            for u in range(2):
                                jt = 2 * g + u
                                nc.tensor.matmul(
                                    pctx[:],
                                    vp_sb[:, jt, h, :],
                                    texp[:, jt, :],
                                    start=(jt == 0),
                                    stop=(jt == NJT - 1),
                                )
                        # reciprocal of row sums, broadcast across partitions
                        trec1 = p2small.tile([1, QCH], F32, tag="trec1")
                        nc.vector.reciprocal(trec1[:], pctx[HD : HD + 1, :])
                        trec = p2small.tile([128, 1, QCH], F32, tag="trec")
                        nc.gpsimd.partition_broadcast(trec[:, 0, :], trec1[:])
                        # normalize attn^T in place (f32r out = rounding write)
                        if VARIANT == "dvenorm":
                            nc.vector.tensor_mul(
                                texp[:],
                                texp[:].bitcast(F32),
                                trec[:].broadcast_to((128, NJT, QCH)),
                            )
                        else:
                            HALF = NJT // 2
                            nc.vector.tensor_mul(
                                texp[:, 0:HALF, :],
                                texp[:, 0:HALF, :].bitcast(F32),
                                trec[:].broadcast_to((128, HALF, QCH)),
                            )
                            nc.gpsimd.tensor_mul(
                                texp[:, HALF:NJT, :],
                                texp[:, HALF:NJT, :].bitcast(F32),
                                trec[:].broadcast_to((128, HALF, QCH)),
                            )
                        # normalized context slice for this head
                        nc.vector.tensor_mul(
                            ctx_sb[h][:, qsl], pctx[0:HD, :], trec[0:HD, 0, :]
                        )
                        if VARIANT == "ctg":
                            attn_dst = attnT_out[h, q]
                        else:
                            attn_dst = attnT_out[h].rearrange(
                                "(jt p) i -> p jt i", p=128
                            )[:, :, qsl]
                        if VARIANT == "dmasplit":
                            HJ = NJT // 2
                            nc.sync.dma_start(
                                attn_dst[:, 0:HJ, :], texp[:, 0:HJ, :].bitcast(F32)
                            )
                            nc.scalar.dma_start(
                                attn_dst[:, HJ:NJT, :],
                                texp[:, HJ:NJT, :].bitcast(F32),
                            )
                        elif VARIANT == "noattn":
                            pass  # timing-only: skip the 64MB attn write
                        else:
                            nc.sync.dma_start(attn_dst, texp[:].bitcast(F32))

                    # output projection for this quarter's i-tiles
                    for jj in range(QCH // 128):
                        it = q * (QCH // 128) + jj
                        isl = slice(it * 128, (it + 1) * 128)
                        o_sb = p3sb.tile([128, D], F32, tag="osb")
                        for nh in range(D // 512):
                            po = pout_pool.tile([128, 512], F32, tag="po")
                            for h in range(HPC):
                                nc.tensor.matmul(
                                    po[:],
                                    ctx_sb[h][:, isl],
                                    wo_sb[:, h, nh * 512 : (nh + 1) * 512],
                                    start=(h == 0),
                                    stop=(h == HPC - 1),
                                )
                            nc.scalar.activation(
                                o_sb[:, nh * 512 : (nh + 1) * 512], po[:], AFT.Copy
                            )
                        nc.gpsimd.dma_start(outp[isl, :], o_sb[:])


def _get_nc():
    if not _NC_CACHE:
        _NC_CACHE.append(_build())
    return _NC_CACHE[0]


def kernel(**inputs):
    q = np.ascontiguousarray(np.asarray(inputs["q"], dtype=np.float32))
    k = np.ascontiguousarray(np.asarray(inputs["k"], dtype=np.float32))
    v = np.ascontiguousarray(np.asarray(inputs["v"], dtype=np.float32))
    Wq = np.asarray(inputs["Wq"], dtype=np.float32)
    Wk = np.asarray(inputs["Wk"], dtype=np.float32)
    Wv = np.asarray(inputs["Wv"], dtype=np.float32)
    Wo = np.asarray(inputs["Wo"], dtype=np.float32)
    bq = np.asarray(inputs["bq"], dtype=np.float32)
    bk = np.asarray(inputs["bk"], dtype=np.float32)
    bv = np.asarray(inputs["bv"], dtype=np.float32)
    bo = np.asarray(inputs["bo"], dtype=np.float32)

    nc = _get_nc()

    qTs = [np.ascontiguousarray(q[b].T) for b in range(B)]
    kTs = [np.ascontiguousarray(k[b].T) for b in range(B)]
    vTs = [np.ascontiguousarray(v[b].T) for b in range(B)]

    in_maps = []
    for core in range(NC):
        b, hb = divmod(core, GPB)
        col0 = hb * COLS
        csl = slice(col0, col0 + COLS)
        in_maps.append(
            {
                "qT": qTs[b],
                "kT": kTs[b],
                "vT": vTs[b],
                "wq": np.ascontiguousarray(Wq[:, csl]),
                "wk": np.ascontiguousarray(Wk[:, csl]),
                "wv": np.ascontiguousarray(Wv[:, csl]),
                "wo": np.ascontiguousarray(Wo[csl, :]),
                "bq": np.ascontiguousarray(bq[csl]).reshape(NM, 128, 1),
                "bk": np.ascontiguousarray(bk[csl]).reshape(NM, 128, 1),
                "bvr": np.ascontiguousarray(bv[csl]).reshape(1, COLS),
            }
        )

    res = run_bass_kernel_spmd(nc, in_maps, core_ids=list(range(NC)))

    attn = np.empty((B, H, S, S), dtype=np.float32)
    out = np.zeros((B, S, D), dtype=np.float32)
    for core in range(NC):
        b, hb = divmod(core, GPB)
        r = res.results[core]
        part = r["attnT_out"]
        for hl in range(HPC):
            attn[b, hb * HPC + hl] = part[hl].T
        out[b] += r["outp"]
    out += bo
    return out, attn
